# revision 15
# baseline (speedup 1.0000x reference)
"""Trainium2 Bass kernel for nn_BinConv2d (XNOR-style binary conv block).

Reference computation (per the oracle):
  h     = batchnorm(x; batch stats over (N,H,W), eps=1e-4, gamma, beta)
  x_bin = sign(h)
  c     = clip(w - mean_c(w), -1, 1); w_bin = sign(c); m_o = mean|c|
  y     = relu((conv2d(x_bin, w_bin, pad=1) + bias) * m_o)

Strategy: data-parallel over batch (4 images per core, 8 cores).
BN statistics are computed on-device per core and combined with a tiny
AllReduce. Both conv operands are exactly +-1 (or 0), so the conv is
computed exactly in fp8e4 with fp32 PSUM accumulation, using DoubleRow
perf mode to contract K=256 per matmul. The 3x3 conv is expressed as 9
shifted accumulating matmuls over a zero-padded SBUF image layout.
"""

import os
import sys

import numpy as np

_TRN_REPO = "/opt/trn_rl_repo"
if _TRN_REPO not in sys.path:
    sys.path.insert(0, _TRN_REPO)

import concourse.bass as bass
import concourse.mybir as mybir
import concourse.tile as tile
from concourse.masks import make_identity

P = 128
C = 256
O = 256
H = W = 56
HW = H * W            # 3136
KH = KW = 3
NKK = KH * KW         # 9
CK = C * NKK          # 2304
RG = 58               # padded row width (1 + 56 + 1)
RPI = 58              # padded rows per image
N_TOTAL = 32
N_CORES = 8
CNT = N_TOTAL * HW    # BN reduction count per channel
BN_EPS = 1e-4
TROWS = 8             # padded output rows per psum tile
NT = H // TROWS       # 7 tiles per image
FREE = TROWS * RG     # 464 (contiguous padded-flat columns, incl pad cols)

F32 = mybir.dt.float32
ALU = mybir.AluOpType
AF = mybir.ActivationFunctionType
AX = mybir.AxisListType


def _legalize_sync_waits(nc, max_waits: int = 1):
    """Work around the ISA's tiny per-instruction sync-wait budgets.

    Tile emits as many semaphore waits per instruction as the dependency
    graph needs, but most walrus instruction formats encode only one sync
    wait ("Too many sync wait commands" codegen failure otherwise).

    Two transformations, both semantics-preserving:
    1. Drop same-engine self-waits that are trivially satisfied: engines
       retire instructions in order, so a wait on the instruction's own
       engine semaphore for a value already reached by preceding
       same-engine updates is a no-op.
    2. For instructions still exceeding `max_waits`, insert a same-engine
       Drain immediately before the offender carrying the excess waits —
       an identical blocking point on the same engine queue (the
       end-of-kernel drain routinely carries 13+ waits, so the Drain
       format is known to have capacity).
    """
    import re

    import bass_rust

    eng_builder = {
        mybir.EngineType.PE: nc.tensor,
        mybir.EngineType.DVE: nc.vector,
        mybir.EngineType.Activation: nc.scalar,
        mybir.EngineType.Pool: nc.gpsimd,
        mybir.EngineType.SP: nc.sync,
    }
    # No self-wait elision: engines pipeline back-to-back instructions, so
    # even same-engine RAW hazards need their semaphore wait (CoreSim race
    # detector confirms). Excess waits are moved to Drain carriers instead.
    self_pat = {}

    def make_drain(engine):
        counts = {id(b): len(b.instructions) for b in nc.main_func.blocks}
        eng_builder[engine].drain()
        for b in nc.main_func.blocks:
            if len(b.instructions) != counts[id(b)]:
                return b.instructions.pop()
        raise RuntimeError("drain emission not found")

    upd: dict = {}
    n_elided = n_moved = 0
    for bb in nc.main_func.blocks:
        out = []
        for ins in bb.instructions:
            si = ins.sync_info
            if si is not None and si.on_wait:
                pat = self_pat.get(ins.engine)
                keep = []
                for w in si.on_wait:
                    if (
                        pat is not None
                        and w.sync_type == "semaphore"
                        and w.wait_mode == "sem-ge-imm"
                        and pat.match(w.ant_name)
                        and upd.get(w.ant_name, 0) >= (w.wait_value or 0)
                    ):
                        n_elided += 1
                        continue
                    keep.append(w)
                while len(keep) > max_waits:
                    dr = make_drain(ins.engine)
                    dr.sync_info = bass_rust.SyncInfo(
                        on_wait=[keep.pop(0)], on_update=[]
                    )
                    out.append(dr)
                    n_moved += 1
                if len(keep) != len(si.on_wait):
                    ins.sync_info = bass_rust.SyncInfo(
                        on_wait=keep, on_update=list(si.on_update)
                    )
            si2 = ins.sync_info
            if si2 is not None:
                for u in si2.on_update:
                    if u.update_mode == "sem-inc":
                        upd[u.ant_name] = upd.get(u.ant_name, 0) + (
                            u.update_value or 1
                        )
            out.append(ins)
        bb.instructions[:] = out
    return n_elided, n_moved


def build_program(nl: int, n_cores: int, use_fp8: bool):
    """Build the SPMD Bass program for `nl` images per core."""
    conv_dt = mybir.dt.float8e4 if use_fp8 else mybir.dt.bfloat16
    perf_mode = mybir.MatmulPerfMode.DoubleRow if use_fp8 else None

    # padded image rows: 1 guard row + nl*58 rows + tail guard, rounded so
    # that ROWS*58 (the DoubleRow j-step in bytes for fp8) is 16-aligned
    rows = 1 + nl * RPI + 1
    while (rows * RG) % 16 != 0:
        rows += 1

    cnt = nl * n_cores * HW  # BN reduction count per channel

    nc = bass.Bass(num_devices=n_cores)

    x_d = nc.declare_dram_parameter("x", [nl, C, H, W], F32, isOutput=False)
    g_d = nc.declare_dram_parameter("gamma", [C], F32, isOutput=False)
    be_d = nc.declare_dram_parameter("beta", [C], F32, isOutput=False)
    w_d = nc.declare_dram_parameter("weight", [O, C, KH, KW], F32, isOutput=False)
    bi_d = nc.declare_dram_parameter("bias", [O], F32, isOutput=False)
    out_d = nc.declare_dram_parameter("out", [nl, O, H, W], F32, isOutput=True)

    replica = [list(range(n_cores))]

    with tile.TileContext(nc) as tc:
        with (
            tc.tile_pool(name="consts", bufs=1) as consts,
            tc.tile_pool(name="xin", bufs=3) as xin_pool,
            tc.tile_pool(name="xbin", bufs=1) as xbin_pool,
            tc.tile_pool(name="wp", bufs=1) as wp,
            tc.tile_pool(name="stat", bufs=1) as stat,
            tc.tile_pool(name="psum", bufs=6, space="PSUM") as psum_pool,
            tc.tile_pool(name="psumt", bufs=2, space="PSUM") as psumt_pool,
            tc.tile_pool(name="osb", bufs=8) as osb_pool,
            tc.tile_pool(name="dram", bufs=1, space="DRAM") as dram_pool,
        ):
            # ---------------- weight prep ----------------
            # natural layout per ogroup: [o_part, (c kh kw)]
            w_nat = []
            for og in range(2):
                t = wp.tile([P, CK], F32, tag=f"wnat{og}")
                nc.sync.dma_start(out=t[:], in_=w_d[og * P:(og + 1) * P, :, :, :])
                w_nat.append(t)

            # per-(kh,kw) mean over input channels, then centered values in
            # (k, c) layout, sign -> conv dtype, clip+abs-sum -> scale
            tr_dt = mybir.dt.bfloat16
            identity = consts.tile([P, P], tr_dt)
            make_identity(nc, identity)

            # wT layout: [c_part, cgroup_j, (k, og) blocks of 128 o-cols]
            wT = wp.tile([P, 2, 18 * P], conv_dt)

            escale = consts.tile([P, 2], F32)   # m/2304 per (o_part, og)
            ebias = consts.tile([P, 2], F32)    # escale * bias
            bias_sb = consts.tile([P, 2], F32)
            nc.sync.dma_start(
                out=bias_sb[:], in_=bi_d[:].rearrange("(a p) -> p a", a=2, p=P)
            )

            for og in range(2):
                wv = w_nat[og].rearrange("p (c k) -> p k c", c=C, k=NKK)
                kmean = stat.tile([P, NKK], F32, tag=f"kmean{og}")
                nc.vector.tensor_reduce(
                    out=kmean[:], in_=wv, axis=AX.X, op=ALU.add
                )
                nc.vector.tensor_scalar_mul(
                    out=kmean[:], in0=kmean[:], scalar1=1.0 / C
                )
                cent = wp.tile([P, CK], F32, tag=f"cent{og}")  # (k, c) layout
                for k in range(NKK):
                    nc.vector.tensor_scalar(
                        out=cent[:, k * C:(k + 1) * C],
                        in0=wv[:, k, :],
                        scalar1=kmean[:, k:k + 1],
                        scalar2=None,
                        op0=ALU.subtract,
                    )
                sgn = wp.tile([P, CK], tr_dt, tag=f"sgn{og}")
                nc.scalar.activation(out=sgn[:], in_=cent[:], func=AF.Sign)
                # clip to [-1,1] (fresh tile: avoid WAR with the Sign read
                # that would add a second sync wait on a 1-slot DVE format),
                # then sum |.|
                clp = wp.tile([P, CK], F32, tag=f"clip{og}")
                nc.vector.tensor_scalar(
                    out=clp[:], in0=cent[:],
                    scalar1=-1.0, scalar2=1.0, op0=ALU.max, op1=ALU.min,
                )
                mraw = stat.tile([P, 1], F32, tag=f"mraw{og}")
                nc.vector.tensor_reduce(
                    out=mraw[:], in_=clp[:], axis=AX.X, op=ALU.add,
                    apply_absolute_value=True,
                )
                nc.vector.tensor_scalar_mul(
                    out=escale[:, og:og + 1], in0=mraw[:], scalar1=1.0 / CK
                )
                # ebias = escale * bias  (on ACT: DVE tensor_tensor has only
                # one sync-wait slot and this op needs DMA + DVE deps)
                nc.scalar.activation(
                    out=ebias[:, og:og + 1],
                    in_=bias_sb[:, og:og + 1],
                    func=AF.Copy,
                    scale=escale[:, og:og + 1],
                )
                # transpose each [o=128, c=128] block into [c, o]
                for k in range(NKK):
                    for cg in range(2):
                        pt = psumt_pool.tile([P, P], tr_dt, tag="pt")
                        nc.tensor.transpose(
                            out=pt[:],
                            in_=sgn[:, k * C + cg * P: k * C + (cg + 1) * P],
                            identity=identity[:],
                        )
                        nc.vector.tensor_copy(
                            out=wT[:, cg, (k * 2 + og) * P:(k * 2 + og + 1) * P],
                            in_=pt[:],
                        )

            # ---------------- BN stats over local shard ----------------
            xsum = stat.tile([P, 2, nl], F32)
            xss = stat.tile([P, 2, nl], F32)
            scr = stat.tile([P, HW], F32)
            for img in range(nl):
                for cg in range(2):
                    xt = xin_pool.tile([P, H, W], F32, tag="xt")
                    nc.sync.dma_start(
                        out=xt[:], in_=x_d[img, cg * P:(cg + 1) * P, :, :]
                    )
                    nc.vector.tensor_reduce(
                        out=xsum[:, cg, img:img + 1],
                        in_=xt.rearrange("p h w -> p (h w)"),
                        axis=AX.X, op=ALU.add,
                    )
                    nc.scalar.activation(
                        out=scr[:],
                        in_=xt.rearrange("p h w -> p (h w)"),
                        func=AF.Square,
                        accum_out=xss[:, cg, img:img + 1],
                    )
            tloc = stat.tile([P, 4], F32)  # [sum_cg0, sum_cg1, ss_cg0, ss_cg1]
            nc.vector.tensor_reduce(
                out=tloc[:, 0:2], in_=xsum[:], axis=AX.X, op=ALU.add
            )
            nc.vector.tensor_reduce(
                out=tloc[:, 2:4], in_=xss[:], axis=AX.X, op=ALU.add
            )

            # ---------------- AllReduce of stats ----------------
            ar_in = dram_pool.tile([4, P], F32)
            ar_out = dram_pool.tile([4, P], F32)
            nc.sync.dma_start(
                out=ar_in[:].rearrange("a p -> p a"), in_=tloc[:]
            )
            if n_cores > 1:
                nc.gpsimd.collective_compute(
                    "AllReduce",
                    ALU.add,
                    replica_groups=replica,
                    ins=[ar_in[:]],
                    outs=[ar_out[:]],
                )
            else:
                nc.gpsimd.dma_start(out=ar_out[:], in_=ar_in[:])
            gstat = stat.tile([P, 4], F32)
            nc.sync.dma_start(
                out=gstat[:], in_=ar_out[:].rearrange("a p -> p a")
            )

            # ---------------- BN affine coefficients ----------------
            # a = gamma * rsqrt(var + eps); b = beta - mean * a
            mean = stat.tile([P, 2], F32)
            nc.vector.tensor_scalar_mul(
                out=mean[:], in0=gstat[:, 0:2], scalar1=1.0 / cnt
            )
            var = stat.tile([P, 2], F32)
            nc.vector.tensor_mul(out=var[:], in0=mean[:], in1=mean[:])
            ex2 = stat.tile([P, 2], F32)
            nc.vector.tensor_scalar_mul(
                out=ex2[:], in0=gstat[:, 2:4], scalar1=1.0 / cnt
            )
            nc.vector.tensor_sub(out=var[:], in0=ex2[:], in1=var[:])
            eps_ap = stat.tile([P, 1], F32)
            nc.vector.memset(eps_ap[:], BN_EPS)
            stdv = stat.tile([P, 2], F32)
            nc.scalar.activation(
                out=stdv[:], in_=var[:], func=AF.Sqrt, bias=eps_ap[:]
            )
            rinv = stat.tile([P, 2], F32)
            nc.vector.reciprocal(out=rinv[:], in_=stdv[:])
            gam2 = stat.tile([P, 2], F32)
            nc.sync.dma_start(
                out=gam2[:], in_=g_d[:].rearrange("(a p) -> p a", a=2, p=P)
            )
            bet2 = stat.tile([P, 2], F32)
            nc.sync.dma_start(
                out=bet2[:], in_=be_d[:].rearrange("(a p) -> p a", a=2, p=P)
            )
            # a = gamma * rinv ; b = beta - mean * a — computed on ACT:
            # these ops mix DMA-loaded params with DVE results, and DVE
            # tensor_tensor formats only encode one sync wait.
            a_t = stat.tile([P, 2], F32)
            b_t = stat.tile([P, 2], F32)
            ma_t = stat.tile([P, 2], F32)
            for cg in range(2):
                nc.scalar.activation(
                    out=a_t[:, cg:cg + 1], in_=rinv[:, cg:cg + 1],
                    func=AF.Copy, scale=gam2[:, cg:cg + 1],
                )
                nc.scalar.activation(
                    out=ma_t[:, cg:cg + 1], in_=mean[:, cg:cg + 1],
                    func=AF.Copy, scale=a_t[:, cg:cg + 1],
                )
                nc.scalar.activation(
                    out=b_t[:, cg:cg + 1], in_=ma_t[:, cg:cg + 1],
                    func=AF.Identity, scale=-1.0, bias=bet2[:, cg:cg + 1],
                )

            # ---------------- binarize x + conv ----------------
            # padded x_bin buffer: [c_part, cgroup_j, padded rows, 58]
            xbin = xbin_pool.tile([P, 2, rows, RG], conv_dt)
            nc.gpsimd.memset(xbin[:], 0.0)

            for img in range(nl):
                r_img = 1 + img * RPI  # first padded row of this image
                for cg in range(2):
                    xt = xin_pool.tile([P, H, W], F32, tag="xt")
                    nc.sync.dma_start(
                        out=xt[:], in_=x_d[img, cg * P:(cg + 1) * P, :, :]
                    )
                    nc.scalar.activation(
                        out=xbin[:, cg, r_img + 1: r_img + 1 + H, 1:1 + W],
                        in_=xt[:],
                        func=AF.Sign,
                        scale=a_t[:, cg:cg + 1],
                        bias=b_t[:, cg:cg + 1],
                    )
                xflat = xbin.rearrange("p j r g -> p j (r g)")
                for og in range(2):
                    for t in range(NT):
                        # output tile: padded rows [pr0, pr0+8) of this image
                        pr0 = r_img + 1 + t * TROWS
                        q0 = pr0 * RG
                        ps = psum_pool.tile([P, FREE], F32, tag="ps")
                        ki = 0
                        for dh in range(3):
                            for dw in range(3):
                                qin = q0 + (dh - 1) * RG + (dw - 1)
                                blk = ((dh * 3 + dw) * 2 + og) * P
                                if use_fp8:
                                    nc.tensor.matmul(
                                        ps[:],
                                        lhsT=wT[:, :, blk:blk + P],
                                        rhs=xflat[:, :, qin:qin + FREE],
                                        start=(ki == 0),
                                        stop=(ki == NKK - 1),
                                        perf_mode=perf_mode,
                                    )
                                else:
                                    for cg in range(2):
                                        nc.tensor.matmul(
                                            ps[:],
                                            lhsT=wT[:, cg, blk:blk + P],
                                            rhs=xflat[:, cg, qin:qin + FREE],
                                            start=(ki == 0 and cg == 0),
                                            stop=(ki == NKK - 1 and cg == 1),
                                        )
                                ki += 1
                        ob = osb_pool.tile([P, TROWS, RG], F32, tag="ob")
                        nc.scalar.activation(
                            out=ob[:],
                            in_=ps[:].rearrange("p (r g) -> p r g", r=TROWS),
                            func=AF.Relu,
                            scale=escale[:, og:og + 1],
                            bias=ebias[:, og:og + 1],
                        )
                        nc.sync.dma_start(
                            out=out_d[img, og * P:(og + 1) * P,
                                      t * TROWS:(t + 1) * TROWS, :],
                            in_=ob[:, :, 1:1 + W],
                        )

    n_elided, n_moved = _legalize_sync_waits(nc)
    return nc


def kernel(**inputs: np.ndarray) -> np.ndarray:
    from concourse.bass_utils import run_bass_kernel_spmd

    x = np.ascontiguousarray(inputs["x"], dtype=np.float32)
    gamma = np.ascontiguousarray(inputs["gamma"], dtype=np.float32)
    beta = np.ascontiguousarray(inputs["beta"], dtype=np.float32)
    weight = np.ascontiguousarray(inputs["weight"], dtype=np.float32)
    bias = np.ascontiguousarray(inputs["bias"], dtype=np.float32)

    n = x.shape[0]
    nl = n // N_CORES
    nc = build_program(nl, N_CORES, use_fp8=True)

    in_maps = []
    for core in range(N_CORES):
        in_maps.append({
            "x": x[core * nl:(core + 1) * nl],
            "gamma": gamma,
            "beta": beta,
            "weight": weight,
            "bias": bias,
        })
    res = run_bass_kernel_spmd(nc, in_maps, list(range(N_CORES)))
    out = np.concatenate([r["out"] for r in res.results], axis=0)
    return out.astype(np.float32)


if __name__ == "__main__":
    # smoke test with random data
    rng = np.random.default_rng(0)
    inputs = {
        "x": rng.standard_normal((32, C, H, W), dtype=np.float32),
        "gamma": np.ones((C,), np.float32),
        "beta": np.zeros((C,), np.float32),
        "weight": (rng.standard_normal((O, C, KH, KW)) * 0.1).astype(np.float32),
        "bias": (rng.standard_normal((O,)) * 0.01).astype(np.float32),
    }
    out = kernel(**inputs)
    print(out.shape, out.dtype, float(np.abs(out).max()))
